# revision 55
# baseline (speedup 1.0000x reference)
"""Trainium2 Bass kernel for sparse multi-headed attention.

Semantics (verified against the reference):
  q = x_q @ Wq.T + bq (per head, dk=32), same for k, v
  for each row s: attend to keys {s-c : c in (5,3,1,0), c <= s}
    score_c[s] = q[s].k[s-c] / sqrt(4)
    p = softmax over valid offsets
    attn[s] = sum_c p_c[s] * v[s-c]
  y = attn @ Wo.T + bo

Sharding: data-parallel over d_stock (8 stocks -> 8 cores). Each core
processes 4 (stock,batch) pairs = 2048 rows. Weights replicated.

v2 design (vs the tf32 baseline):
  - fp16 activations/weights on the wire and in matmuls (10-bit mantissa,
    ~tf32 accuracy, half the HBM traffic, fast PE weight loads), bf16 for
    everything post-exp (range safety: exp(score) can exceed fp16 max).
  - All DVE tensor-tensor ops use 16-bit dtypes -> 2x DVE mode.
  - The p head->feature broadcast runs on the DMA engines via a DRAM
    round-trip with a stride-0 source AP (frees the PE and keeps the
    attend multiplies in 2x mode), not on the PE.
  - Output is written feature-major (y^T) and transposed on the host.
  - Keep-alive matmuls bridge the softmax gap so the PE HAM clock gate
    never re-throttles to half speed.
"""

import numpy as np

from concourse import bacc, bass, mybir, tile
from concourse.bass_utils import run_bass_kernel_spmd

DS, NB, S, DM, H, DK = 8, 4, 512, 256, 8, 32
CONS = (5, 3, 1, 0)
NCORES = 8
NPAIR = NB  # pairs per core (1 stock x 4 batches)
ROWS = NPAIR * S  # 2048
P = 128
PADC = 8  # zero pad columns in front of k/v for shifted reads
NEG = -1e9
SCALE = 0.5  # 1/sqrt(n_att)

f32 = mybir.dt.float32
f16 = mybir.dt.float16
bf16 = mybir.dt.bfloat16
Act = mybir.ActivationFunctionType
Alu = mybir.AluOpType


def _ap(base, off_elems, dims):
    return bass.AP(tensor=base.tensor, offset=base.offset + off_elems, ap=dims)


def _emit(ctx, tc, nc, d, y_dram):
    main = ctx.enter_context(tc.tile_pool(name="main", bufs=1))
    prodp = ctx.enter_context(tc.tile_pool(name="prodp", bufs=4))
    utp = ctx.enter_context(tc.tile_pool(name="utp", bufs=8))
    yst = ctx.enter_context(tc.tile_pool(name="yst", bufs=2))
    pj = ctx.enter_context(tc.tile_pool(name="pj", bufs=2, space="PSUM"))
    scp = ctx.enter_context(tc.tile_pool(name="scp", bufs=1, space="PSUM"))
    dramp = ctx.enter_context(tc.tile_pool(name="dramp", bufs=1, space="DRAM"))

    # ---------------- input DMAs (SP queue, in issue order) ------------
    ws = [main.tile([P, 4 * DM], f16, name=f"ws{k}") for k in range(2)]
    for k in range(2):
        nc.sync.dma_start(out=ws[k][:], in_=d["w"][k * P:(k + 1) * P, :])
    bsb = [main.tile([P, 4], f32, name=f"bsb{ch}") for ch in range(2)]
    selk = [main.tile([P, 224], f16, name=f"selk{ch}") for ch in range(2)]
    selbc = main.tile([32, 8 * P], f16, name="selbc")
    xs = {}
    for name in ("xq", "xk", "xv"):
        for k in range(2):
            xs[name, k] = main.tile([P, ROWS], f16, name=f"{name}{k}")
    # halves of q/k first so projections start early; v afterwards.
    # Spread triggers over the two HWDGE queues so they don't serialize.
    for k in range(2):
        nc.sync.dma_start(out=xs["xq", k][:, 0:1024],
                          in_=d["xq"][k * P:(k + 1) * P, 0:1024])
        nc.scalar.dma_start(out=xs["xk", k][:, 0:1024],
                            in_=d["xk"][k * P:(k + 1) * P, 0:1024])
    for k in range(2):
        nc.sync.dma_start(out=xs["xq", k][:, 1024:2048],
                          in_=d["xq"][k * P:(k + 1) * P, 1024:2048])
        nc.scalar.dma_start(out=xs["xk", k][:, 1024:2048],
                            in_=d["xk"][k * P:(k + 1) * P, 1024:2048])
    nc.scalar.dma_start(out=selbc[:], in_=d["selbc"])
    for ch in range(2):
        nc.scalar.dma_start(out=bsb[ch][:],
                            in_=d["bqkvo"][ch * P:(ch + 1) * P, :])
        nc.scalar.dma_start(out=selk[ch][:],
                            in_=d["selkm"][ch * P:(ch + 1) * P, :])
    for k in range(2):
        nc.sync.dma_start(out=xs["xv", k][:],
                          in_=d["xv"][k * P:(k + 1) * P, :])

    # ---------------- PE warmup (HAM un-throttle) while DMAs run ------
    wtb = main.tile([P, 512], bf16, name="wtb")
    wtf = main.tile([P, 512], f16, name="wtf")
    nc.vector.memset(wtb[:], 0.0)
    nc.vector.memset(wtf[:], 0.0)
    for i in range(4):
        wps = pj.tile([P, 1024], f32, name="wps", tag="pj")
        nc.tensor.matmul(wps[:, 0:512], lhsT=wtb[:, 0:P], rhs=wtb[:],
                         start=True, stop=True)

    # ---------------- projections -------------------------------------
    q_sb = [main.tile([P, ROWS], f16, name=f"q{ch}") for ch in range(2)]
    k_sb = [main.tile([P, PADC + ROWS], f16, name=f"k{ch}") for ch in range(2)]
    v_sb = [main.tile([P, PADC + ROWS], bf16, name=f"v{ch}") for ch in range(2)]
    for ch in range(2):
        nc.vector.memset(k_sb[ch][:, 0:PADC], 0.0)
        nc.vector.memset(v_sb[ch][:, 0:PADC], 0.0)

    def project(name, bcol, ch, h):
        ps = pj.tile([P, 1024], f32, name="ps", tag="pj")
        for k in range(2):
            lhsT = ws[k][:, bcol * DM + ch * P: bcol * DM + (ch + 1) * P]
            for hh2 in range(2):  # fp16 moving operand max is 512
                nc.tensor.matmul(
                    ps[:, hh2 * 512:(hh2 + 1) * 512],
                    lhsT=lhsT,
                    rhs=xs[name, k][:, h * 1024 + hh2 * 512:
                                    h * 1024 + (hh2 + 1) * 512],
                    start=(k == 0), stop=(k == 1))
        # biases fold into the PSUM->SBUF copies: q/v on ACT (activation
        # bias), k on DVE (tensor_scalar add) to balance engine load
        bias_ap = bsb[ch][:, bcol:bcol + 1]
        if name == "xq":
            out = q_sb[ch][:, h * 1024:(h + 1) * 1024]
        elif name == "xk":
            out = k_sb[ch][:, PADC + h * 1024: PADC + (h + 1) * 1024]
        else:
            out = v_sb[ch][:, PADC + h * 1024: PADC + (h + 1) * 1024]
        if name != "xv" and h == 0:
            # half-column copies so pair-0 products start one copy sooner
            nc.scalar.activation(out[:, 0:512], ps[:, 0:512], Act.Identity,
                                 bias=bias_ap)
            nc.scalar.activation(out[:, 512:1024], ps[:, 512:1024],
                                 Act.Identity, bias=bias_ap)
        else:
            nc.scalar.activation(out, ps[:], Act.Identity, bias=bias_ap)

    for h in range(2):
        for ch in range(2):
            project("xq", 0, ch, h)
            project("xk", 1, ch, h)
    for h in range(2):
        for ch in range(2):
            project("xv", 2, ch, h)

    # ---------------- products (DVE) + scores (PE), cig-split ----------
    # pr[(p,ch,cig)][d, j, s] = q[d, p*512+s] * k[d, p*512+s-c(cig,j)]
    # sc[8*hh + 4*ch + p, ci*512 + s] = q_h . k_h(s-c) * 0.5  (h = 4ch+hh)
    # cig0 products/scores complete first so exp(half 0) overlaps the
    # cig1 score matmuls.
    sc = scp.tile([P, 4 * 512], f32, name="sc", tag="sc")
    p_sb = main.tile([P, 4, 512], bf16, name="p_sb")
    d2 = main.tile([P, 2, 512], bf16, name="d2")

    def products(cig, step):
        c_hi = CONS[cig]
        for p in range(NPAIR):
            for ch in range(2):
                pr = prodp.tile([P, 2, 512], f16, name="pr", tag="pr")
                q_ap = q_sb[ch][:, p * 512:(p + 1) * 512]
                q_b = _ap(q_ap, 0, [q_ap.ap[0], [0, 2], [1, 512]])
                k_ap = k_sb[ch][:]
                k_v = _ap(k_ap, PADC + p * 512 - c_hi,
                          [k_ap.ap[0], [step, 2], [1, 512]])
                # flat out view: a 3D out AP drops DVE to 1x mode
                pr_flat = _ap(pr[:], 0, [pr[:].ap[0], [1, 1024]])
                nc.vector.tensor_tensor(out=pr_flat, in0=k_v, in1=q_b,
                                        op=Alu.mult)
                prs[p, ch, cig] = pr

    def score_mms(cig):
        for p in range(NPAIR):
            for ch in range(2):
                lhsT = selk[ch][:, 96 - p: 224 - p]
                for j in range(2):
                    nc.tensor.matmul(
                        sc[:, (cig + j) * 512:(cig + j + 1) * 512],
                        lhsT=lhsT,
                        rhs=prs[p, ch, cig][:, j, :],
                        start=(p == 0 and ch == 0),
                        stop=(p == 3 and ch == 1))

    def exp_half(h):
        nc.scalar.activation(
            _ap(p_sb[:], h * 1024, [p_sb[:].ap[0], [1, 1024]]),
            sc[:, h * 1024:(h + 1) * 1024], Act.Exp)
        nc.vector.tensor_tensor(out=d2[:, h, :], in0=p_sb[:, 2 * h, :],
                                in1=p_sb[:, 2 * h + 1, :], op=Alu.add)

    prs = {}
    products(0, 2)
    score_mms(0)
    products(2, 1)  # before the memsets so the DVE doesn't stall on them
    for ci, c in ((0, 5), (1, 3)):
        nc.vector.memset(sc[:, ci * 512: ci * 512 + c], NEG)
    exp_half(0)
    score_mms(2)
    nc.vector.memset(sc[:, 2 * 512: 2 * 512 + 1], NEG)
    exp_half(1)

    # ---------------- softmax tail -------------------------------------
    den = main.tile([P, 512], f32, name="den")
    nc.vector.tensor_tensor(out=den[:], in0=d2[:, 0, :], in1=d2[:, 1, :],
                            op=Alu.add)
    rcpf = main.tile([P, 512], f32, name="rcpf")
    nc.vector.reciprocal_approx_fast(rcpf[:], den[:])
    rcp = main.tile([P, 512], bf16, name="rcp")
    nc.vector.tensor_copy(rcp[:], rcpf[:])
    for h in range(2):
        pv = _ap(p_sb[:], h * 1024, [p_sb[:].ap[0], [1, 1024]])
        pv3 = _ap(p_sb[:], h * 1024, [p_sb[:].ap[0], [512, 2], [1, 512]])
        r_b = _ap(rcp[:], 0, [rcp[:].ap[0], [0, 2], [1, 512]])
        nc.vector.tensor_tensor(out=pv, in0=pv3, in1=r_b, op=Alu.mult)

    # ---------------- p broadcast via DMA round-trip -------------------
    # pscr row 8*hh + 4*ch + p = sc lane; bc[d, ch*8192 + p*2048 + ci*512+s]
    # cig0 (offsets 5,3) is broadcast on the PE: one [32,128]-selector
    # matmul per (p, ch) with a partition-offset rhs slice of p_sb — no
    # DMA latency, attend starts right after pmult half 0.
    # cig1 (offsets 1,0) takes the DRAM round-trip, whose latency hides
    # under the cig0 attend work.
    pscr = dramp.tile([32, 1024], bf16, name="pscr")
    bc = main.tile([P, 16384], bf16, name="bc")
    pbase = pscr[:]
    nc.sync.dma_start(out=pscr[:], in_=p_sb[0:32, 2:4, :])
    for hh in range(4):
        src = _ap(pbase, hh * 8 * 1024,
                  [[0, 32], [1024, 8], [1, 1024]])
        dst_base = bc[32 * hh:32 * (hh + 1), :]
        dst = _ap(dst_base, 1024,
                  [dst_base.ap[0], [2048, 8], [1, 1024]])
        # split across the two HWDGE rings so transfers run in parallel
        eng = nc.sync if hh < 2 else nc.scalar
        eng.dma_start(out=dst, in_=src)
    # keep-alives so the HAM clock gate stays open through the softmax
    for rhs in (p_sb[:, 0, :], p_sb[:, 2, :], d2[:, 0, :]):
        ka = pj.tile([P, 1024], f32, name="ka", tag="pj")
        nc.tensor.matmul(ka[:, 0:512], lhsT=wtb[:, 0:P], rhs=rhs,
                         start=True, stop=True)
    # cig0 broadcast lands in PSUM, then ACT (idle here) copies it to
    # SBUF bf16 so the attend multiplies run in the DVE 2x mode.
    bc_ps = {}
    for p in range(NPAIR):
        for ch in range(2):
            bp = pj.tile([P, 1024], f32, name="bp", tag="pj")
            idx = 2 * p + ch
            for ci in range(2):
                nc.tensor.matmul(bp[:, ci * 512:(ci + 1) * 512],
                                 lhsT=selbc[:, idx * P:(idx + 1) * P],
                                 rhs=p_sb[0:32, ci, :],
                                 start=True, stop=True)
            bc_ps[p, ch] = bp

    # ---------------- attend (DVE, 16-bit) -----------------------------
    # ut[ci] = v[d, s-c] * bc;  u2 col p*1024 + j*512 + s, j = cig half
    u2 = [main.tile([P, 4096], f16, name=f"u2{ch}") for ch in range(2)]
    uts = {}
    for cig, step in ((0, 2), (2, 1)):
        c_hi = CONS[cig]
        for p in range(NPAIR):
            for ch in range(2):
                if cig == 0:
                    uts[p, ch] = utp.tile([P, 4, 512], f16, name="ut",
                                          tag="ut")
                ut = uts[p, ch]
                if cig == 0:
                    bc_v = bc_ps[p, ch][:]
                else:
                    bc_v = _ap(bc[:], ch * 8192 + p * 2048 + cig * 512,
                               [bc[:].ap[0], [512, 2], [1, 512]])
                v_ap = v_sb[ch][:]
                v_v = _ap(v_ap, PADC + p * 512 - c_hi,
                          [v_ap.ap[0], [step, 2], [1, 512]])
                outv = _ap(ut[:], cig * 512, [ut[:].ap[0], [1, 1024]])
                nc.vector.tensor_tensor(out=outv, in0=v_v, in1=bc_v,
                                        op=Alu.mult)
                if cig == 2:
                    # cig0's partial sum folds into the output projection;
                    # only the late half keeps a DVE add
                    nc.vector.tensor_tensor(
                        out=u2[ch][:, p * 1024 + 512: p * 1024 + 1024],
                        in0=ut[:, 2, :], in1=ut[:, 3, :], op=Alu.add)

    # ---------------- output projection: y^T = Wo u (+bo) --------------
    for h in range(2):
        for e in range(2):
            yp = pj.tile([P, 1024], f32, name="yp", tag="pj")
            for ch in range(2):
                lhsT = ws[ch][:, 3 * DM + e * P: 3 * DM + (e + 1) * P]
                for st in range(3):  # streams: ut ci0, ut ci1, u2 j1
                    for pp in (2 * h, 2 * h + 1):
                        if st < 2:
                            rhs = uts[pp, ch][:, st, :]
                        else:
                            rhs = u2[ch][:, pp * 1024 + 512:
                                         pp * 1024 + 1024]
                        nc.tensor.matmul(
                            yp[:, (pp - 2 * h) * 512:(pp - 2 * h + 1) * 512],
                            lhsT=lhsT,
                            rhs=rhs,
                            start=(ch == 0 and st == 0),
                            stop=(ch == 1 and st == 2))
            yo = yst.tile([P, 1024], f16, name="yo", tag="yo")
            nc.scalar.activation(yo[:], yp[:], Act.Identity,
                                 bias=bsb[e][:, 3:4])
            nc.scalar.dma_start(
                out=y_dram[e * P:(e + 1) * P, h * 1024:(h + 1) * 1024],
                in_=yo[:])


def build_nc():
    from contextlib import ExitStack
    nc = bacc.Bacc(trn_type="TRN2", target_bir_lowering=False, debug=False)
    d = {}
    for name in ("xq", "xk", "xv"):
        d[name] = nc.dram_tensor(name, [DM, ROWS], f16, kind="ExternalInput").ap()
    d["w"] = nc.dram_tensor("w", [DM, 4 * DM], f16, kind="ExternalInput").ap()
    d["bqkvo"] = nc.dram_tensor("bqkvo", [DM, 4], f32, kind="ExternalInput").ap()
    d["selkm"] = nc.dram_tensor("selkm", [DM, 224], f16, kind="ExternalInput").ap()
    d["selbc"] = nc.dram_tensor("selbc", [32, 8 * P], f16, kind="ExternalInput").ap()
    y = nc.dram_tensor("y", [DM, ROWS], f16, kind="ExternalOutput").ap()
    with tile.TileContext(nc) as tc:
        with ExitStack() as ctx:
            _emit(ctx, tc, nc, d, y)
    nc.compile()
    return nc


def make_shared_inputs(Wq, bq, Wk, bk, Wv, bv, Wo, bo):
    shared = {}
    wt = np.concatenate(
        [np.asarray(W, np.float32).T for W in (Wq, Wk, Wv, Wo)], axis=1)
    shared["w"] = wt.astype(np.float16)
    shared["bqkvo"] = np.ascontiguousarray(
        np.stack([bq, bk, bv, bo], axis=1), dtype=np.float32)
    # selkm[ch*128 + dd, 96 + 8*(dd//32) + 4*ch] = 0.5; score matmul for
    # pair p uses lhsT = selkm[ch][:, 96-p : 224-p] -> head hh of pair p
    # lands on psum lane 8*hh + 4*ch + p.
    selkm = np.zeros((DM, 224), np.float16)
    for ch in range(2):
        for dd in range(P):
            selkm[ch * P + dd, 96 + 8 * (dd // 32) + 4 * ch] = SCALE
    shared["selkm"] = selkm
    # selbc[k, (2p+ch)*128 + d] = 1 iff k == 8*(d//32) + 4*ch + p: the
    # broadcast matmul for (p, ch) maps p_sb lane 8*hh + 4*ch + p to
    # feature lanes 32*hh..32*hh+31 of channel half ch.
    selbc = np.zeros((32, 8 * P), np.float16)
    for p in range(NPAIR):
        for ch in range(2):
            for dd in range(P):
                selbc[8 * (dd // 32) + 4 * ch + p,
                      (2 * p + ch) * P + dd] = 1.0
    shared["selbc"] = selbc
    return shared


def make_core_inputs(query, key_in, value, core):
    # core i handles stock i: [4, 512, 256] -> feature-major [256, 2048]
    out = {}
    for name, x in (("xq", query), ("xk", key_in), ("xv", value)):
        xi = np.asarray(x[core], dtype=np.float32).reshape(ROWS, DM)
        out[name] = np.ascontiguousarray(xi.T).astype(np.float16)
    return out


def kernel(query, key_in, value, Wq, bq, Wk, bk, Wv, bv, Wo, bo):
    nc = build_nc()
    shared = make_shared_inputs(Wq, bq, Wk, bk, Wv, bv, Wo, bo)
    in_maps = []
    for core in range(NCORES):
        m = dict(shared)
        m.update(make_core_inputs(query, key_in, value, core))
        in_maps.append(m)
    res = run_bass_kernel_spmd(nc, in_maps, list(range(NCORES))).results
    # y dram is y^T [256, 2048]; host transposes back to [4, 512, 256]
    y = np.stack([
        np.ascontiguousarray(res[i]["y"].astype(np.float32).T).reshape(
            NB, S, DM)
        for i in range(NCORES)])
    return y.astype(np.float32)


# revision 61
# speedup vs baseline: 1.1116x; 1.1116x over previous
"""Trainium2 Bass kernel for sparse multi-headed attention.

Semantics (verified against the reference):
  q = x_q @ Wq.T + bq (per head, dk=32), same for k, v
  for each row s: attend to keys {s-c : c in (5,3,1,0), c <= s}
    score_c[s] = q[s].k[s-c] / sqrt(4)
    p = softmax over valid offsets
    attn[s] = sum_c p_c[s] * v[s-c]
  y = attn @ Wo.T + bo

Sharding: data-parallel over d_stock (8 stocks -> 8 cores). Each core
processes 4 (stock,batch) pairs = 2048 rows. Weights replicated.

v2 design (vs the tf32 baseline):
  - fp16 activations/weights on the wire and in matmuls (10-bit mantissa,
    ~tf32 accuracy, half the HBM traffic, fast PE weight loads), bf16 for
    everything post-exp (range safety: exp(score) can exceed fp16 max).
  - All DVE tensor-tensor ops use 16-bit dtypes -> 2x DVE mode.
  - The p head->feature broadcast runs on the DMA engines via a DRAM
    round-trip with a stride-0 source AP (frees the PE and keeps the
    attend multiplies in 2x mode), not on the PE.
  - Output is written feature-major (y^T) and transposed on the host.
  - Keep-alive matmuls bridge the softmax gap so the PE HAM clock gate
    never re-throttles to half speed.
"""

import numpy as np

from concourse import bacc, bass, mybir, tile
from concourse.bass_utils import run_bass_kernel_spmd

DS, NB, S, DM, H, DK = 8, 4, 512, 256, 8, 32
CONS = (5, 3, 1, 0)
NCORES = 8
NPAIR = NB  # pairs per core (1 stock x 4 batches)
ROWS = NPAIR * S  # 2048
P = 128
PADC = 8  # zero pad columns in front of k/v for shifted reads
NEG = -1e9
SCALE = 0.5  # 1/sqrt(n_att)

f32 = mybir.dt.float32
f16 = mybir.dt.float16
bf16 = mybir.dt.bfloat16
Act = mybir.ActivationFunctionType
Alu = mybir.AluOpType


def _ap(base, off_elems, dims):
    return bass.AP(tensor=base.tensor, offset=base.offset + off_elems, ap=dims)


def _emit(ctx, tc, nc, d, y_dram):
    main = ctx.enter_context(tc.tile_pool(name="main", bufs=1))
    prodp = ctx.enter_context(tc.tile_pool(name="prodp", bufs=4))
    utp = ctx.enter_context(tc.tile_pool(name="utp", bufs=8))
    yst = ctx.enter_context(tc.tile_pool(name="yst", bufs=2))
    pj = ctx.enter_context(tc.tile_pool(name="pj", bufs=2, space="PSUM"))
    scp = ctx.enter_context(tc.tile_pool(name="scp", bufs=1, space="PSUM"))
    dramp = ctx.enter_context(tc.tile_pool(name="dramp", bufs=1, space="DRAM"))

    # ---------------- input DMAs (SP queue, in issue order) ------------
    ws = [main.tile([P, 4 * DM], f16, name=f"ws{k}") for k in range(2)]
    for k in range(2):
        nc.sync.dma_start(out=ws[k][:], in_=d["w"][k * P:(k + 1) * P, :])
    bsb = [main.tile([P, 4], f32, name=f"bsb{ch}") for ch in range(2)]
    selk = [main.tile([P, 224], f16, name=f"selk{ch}") for ch in range(2)]
    selbc = main.tile([32, 8 * P], f16, name="selbc")
    xs = {}
    for name in ("xq", "xk", "xv"):
        for k in range(2):
            xs[name, k] = main.tile([P, ROWS], f16, name=f"{name}{k}")
    # halves of q/k first so projections start early; v afterwards.
    # Spread triggers over the two HWDGE queues so they don't serialize.
    for k in range(2):
        nc.sync.dma_start(out=xs["xq", k][:, 0:1024],
                          in_=d["xq"][k * P:(k + 1) * P, 0:1024])
        nc.scalar.dma_start(out=xs["xk", k][:, 0:1024],
                            in_=d["xk"][k * P:(k + 1) * P, 0:1024])
    for k in range(2):
        nc.sync.dma_start(out=xs["xq", k][:, 1024:2048],
                          in_=d["xq"][k * P:(k + 1) * P, 1024:2048])
        nc.scalar.dma_start(out=xs["xk", k][:, 1024:2048],
                            in_=d["xk"][k * P:(k + 1) * P, 1024:2048])
    nc.scalar.dma_start(out=selbc[:], in_=d["selbc"])
    for ch in range(2):
        nc.scalar.dma_start(out=bsb[ch][:],
                            in_=d["bqkvo"][ch * P:(ch + 1) * P, :])
        nc.scalar.dma_start(out=selk[ch][:],
                            in_=d["selkm"][ch * P:(ch + 1) * P, :])
    for k in range(2):
        nc.sync.dma_start(out=xs["xv", k][:],
                          in_=d["xv"][k * P:(k + 1) * P, :])

    # ---------------- PE warmup (HAM un-throttle) while DMAs run ------
    wtb = main.tile([P, 512], bf16, name="wtb")
    wtf = main.tile([P, 512], f16, name="wtf")
    nc.vector.memset(wtb[:], 0.0)
    nc.vector.memset(wtf[:], 0.0)
    # 6 x ~790ns cold crosses the 3.413us HAM window, so the PE
    # un-throttles before the projections (4 x was just short: the whole
    # front ran at 1.2GHz per the HAM event trace)
    for i in range(6):
        wps = pj.tile([P, 1024], f32, name="wps", tag="pj")
        nc.tensor.matmul(wps[:, 0:512], lhsT=wtb[:, 0:P], rhs=wtb[:],
                         start=True, stop=True)

    # ---------------- projections -------------------------------------
    q_sb = [main.tile([P, ROWS], f16, name=f"q{ch}") for ch in range(2)]
    k_sb = [main.tile([P, PADC + ROWS], f16, name=f"k{ch}") for ch in range(2)]
    v_sb = [main.tile([P, PADC + ROWS], bf16, name=f"v{ch}") for ch in range(2)]
    for ch in range(2):
        nc.vector.memset(k_sb[ch][:, 0:PADC], 0.0)
        nc.vector.memset(v_sb[ch][:, 0:PADC], 0.0)

    def project(name, bcol, ch, h):
        ps = pj.tile([P, 1024], f32, name="ps", tag="pj")
        for k in range(2):
            lhsT = ws[k][:, bcol * DM + ch * P: bcol * DM + (ch + 1) * P]
            for hh2 in range(2):  # fp16 moving operand max is 512
                nc.tensor.matmul(
                    ps[:, hh2 * 512:(hh2 + 1) * 512],
                    lhsT=lhsT,
                    rhs=xs[name, k][:, h * 1024 + hh2 * 512:
                                    h * 1024 + (hh2 + 1) * 512],
                    start=(k == 0), stop=(k == 1))
        # biases fold into the PSUM->SBUF copies: q/v on ACT (activation
        # bias), k on DVE (tensor_scalar add) to balance engine load
        bias_ap = bsb[ch][:, bcol:bcol + 1]
        if name == "xq":
            out = q_sb[ch][:, h * 1024:(h + 1) * 1024]
        elif name == "xk":
            out = k_sb[ch][:, PADC + h * 1024: PADC + (h + 1) * 1024]
        else:
            out = v_sb[ch][:, PADC + h * 1024: PADC + (h + 1) * 1024]
        if name != "xv" and h == 0:
            # half-column copies so pair-0 products start one copy sooner
            nc.scalar.activation(out[:, 0:512], ps[:, 0:512], Act.Identity,
                                 bias=bias_ap)
            nc.scalar.activation(out[:, 512:1024], ps[:, 512:1024],
                                 Act.Identity, bias=bias_ap)
        else:
            nc.scalar.activation(out, ps[:], Act.Identity, bias=bias_ap)

    for h in range(2):
        for ch in range(2):
            project("xq", 0, ch, h)
            project("xk", 1, ch, h)
    for h in range(2):
        for ch in range(2):
            project("xv", 2, ch, h)

    # ---------------- products (DVE) + scores (PE), cig-split ----------
    # pr[(p,ch,cig)][d, j, s] = q[d, p*512+s] * k[d, p*512+s-c(cig,j)]
    # sc[8*hh + 4*ch + p, ci*512 + s] = q_h . k_h(s-c) * 0.5  (h = 4ch+hh)
    # cig0 products/scores complete first so exp(half 0) overlaps the
    # cig1 score matmuls.
    sc = scp.tile([P, 4 * 512], f32, name="sc", tag="sc")
    p_sb = main.tile([P, 4, 512], bf16, name="p_sb")
    d2 = main.tile([P, 2, 512], bf16, name="d2")

    def products(cig, step):
        c_hi = CONS[cig]
        for p in range(NPAIR):
            for ch in range(2):
                pr = prodp.tile([P, 2, 512], f16, name="pr", tag="pr")
                q_ap = q_sb[ch][:, p * 512:(p + 1) * 512]
                q_b = _ap(q_ap, 0, [q_ap.ap[0], [0, 2], [1, 512]])
                k_ap = k_sb[ch][:]
                k_v = _ap(k_ap, PADC + p * 512 - c_hi,
                          [k_ap.ap[0], [step, 2], [1, 512]])
                # flat out view: a 3D out AP drops DVE to 1x mode
                pr_flat = _ap(pr[:], 0, [pr[:].ap[0], [1, 1024]])
                nc.vector.tensor_tensor(out=pr_flat, in0=k_v, in1=q_b,
                                        op=Alu.mult)
                prs[p, ch, cig] = pr

    def score_mms(cig):
        for p in range(NPAIR):
            for ch in range(2):
                lhsT = selk[ch][:, 96 - p: 224 - p]
                for j in range(2):
                    nc.tensor.matmul(
                        sc[:, (cig + j) * 512:(cig + j + 1) * 512],
                        lhsT=lhsT,
                        rhs=prs[p, ch, cig][:, j, :],
                        start=(p == 0 and ch == 0),
                        stop=(p == 3 and ch == 1))

    def exp_half(h):
        nc.scalar.activation(
            _ap(p_sb[:], h * 1024, [p_sb[:].ap[0], [1, 1024]]),
            sc[:, h * 1024:(h + 1) * 1024], Act.Exp)
        nc.vector.tensor_tensor(out=d2[:, h, :], in0=p_sb[:, 2 * h, :],
                                in1=p_sb[:, 2 * h + 1, :], op=Alu.add)

    prs = {}
    products(0, 2)
    score_mms(0)
    products(2, 1)  # before the memsets so the DVE doesn't stall on them
    for ci, c in ((0, 5), (1, 3)):
        nc.vector.memset(sc[:, ci * 512: ci * 512 + c], NEG)
    exp_half(0)
    score_mms(2)
    nc.vector.memset(sc[:, 2 * 512: 2 * 512 + 1], NEG)
    exp_half(1)

    # ---------------- softmax tail -------------------------------------
    den = main.tile([P, 512], f32, name="den")
    nc.vector.tensor_tensor(out=den[:], in0=d2[:, 0, :], in1=d2[:, 1, :],
                            op=Alu.add)
    rcpf = main.tile([P, 512], f32, name="rcpf")
    nc.vector.reciprocal_approx_fast(rcpf[:], den[:])
    rcp = main.tile([P, 512], bf16, name="rcp")
    nc.vector.tensor_copy(rcp[:], rcpf[:])
    for h in range(2):
        pv = _ap(p_sb[:], h * 1024, [p_sb[:].ap[0], [1, 1024]])
        pv3 = _ap(p_sb[:], h * 1024, [p_sb[:].ap[0], [512, 2], [1, 512]])
        r_b = _ap(rcp[:], 0, [rcp[:].ap[0], [0, 2], [1, 512]])
        nc.vector.tensor_tensor(out=pv, in0=pv3, in1=r_b, op=Alu.mult)

    # ---------------- p broadcast via DMA round-trip -------------------
    # pscr row 8*hh + 4*ch + p = sc lane; bc[d, ch*8192 + p*2048 + ci*512+s]
    # cig0 (offsets 5,3) is broadcast on the PE: one [32,128]-selector
    # matmul per (p, ch) with a partition-offset rhs slice of p_sb — no
    # DMA latency, attend starts right after pmult half 0.
    # cig1 (offsets 1,0) takes the DRAM round-trip, whose latency hides
    # under the cig0 attend work.
    pscr = dramp.tile([32, 1024], bf16, name="pscr")
    bc = main.tile([P, 16384], bf16, name="bc")
    pbase = pscr[:]
    nc.sync.dma_start(out=pscr[:], in_=p_sb[0:32, 2:4, :])
    for hh in range(4):
        src = _ap(pbase, hh * 8 * 1024,
                  [[0, 32], [1024, 8], [1, 1024]])
        dst_base = bc[32 * hh:32 * (hh + 1), :]
        dst = _ap(dst_base, 1024,
                  [dst_base.ap[0], [2048, 8], [1, 1024]])
        # split across the two HWDGE rings so transfers run in parallel
        eng = nc.sync if hh < 2 else nc.scalar
        eng.dma_start(out=dst, in_=src)
    # dense bridge right after the scores: ~3.5us of continuous dummy
    # matmuls carries the HAM clock gate through the softmax latency
    # (the gate dropped at scores-end + 3.4us in the best trace)
    for i in range(16):
        wps = pj.tile([P, 1024], f32, name="wps", tag="pj")
        nc.tensor.matmul(wps[:, 0:512], lhsT=wtb[:, 0:P], rhs=wtb[:],
                         start=True, stop=True)
    # keep-alives so the HAM clock gate stays open through the softmax
    for rhs in (p_sb[:, 0, :], p_sb[:, 2, :], d2[:, 0, :]):
        ka = pj.tile([P, 1024], f32, name="ka", tag="pj")
        nc.tensor.matmul(ka[:, 0:512], lhsT=wtb[:, 0:P], rhs=rhs,
                         start=True, stop=True)
    # cig0 broadcast lands in PSUM, then ACT (idle here) copies it to
    # SBUF bf16 so the attend multiplies run in the DVE 2x mode.
    bc_ps = {}
    for p in range(NPAIR):
        for ch in range(2):
            bp = pj.tile([P, 1024], f32, name="bp", tag="pj")
            idx = 2 * p + ch
            for ci in range(2):
                nc.tensor.matmul(bp[:, ci * 512:(ci + 1) * 512],
                                 lhsT=selbc[:, idx * P:(idx + 1) * P],
                                 rhs=p_sb[0:32, ci, :],
                                 start=True, stop=True)
            bc_ps[p, ch] = bp

    # ---------------- attend (DVE, 16-bit) -----------------------------
    # ut[ci] = v[d, s-c] * bc;  u2 col p*1024 + j*512 + s, j = cig half
    u2 = [main.tile([P, 4096], f16, name=f"u2{ch}") for ch in range(2)]
    uts = {}
    for cig, step in ((0, 2), (2, 1)):
        c_hi = CONS[cig]
        for p in range(NPAIR):
            for ch in range(2):
                if cig == 0:
                    uts[p, ch] = utp.tile([P, 4, 512], bf16, name="ut",
                                          tag="ut")
                ut = uts[p, ch]
                if cig == 0:
                    bc_v = bc_ps[p, ch][:]
                else:
                    bc_v = _ap(bc[:], ch * 8192 + p * 2048 + cig * 512,
                               [bc[:].ap[0], [512, 2], [1, 512]])
                v_ap = v_sb[ch][:]
                v_v = _ap(v_ap, PADC + p * 512 - c_hi,
                          [v_ap.ap[0], [step, 2], [1, 512]])
                outv = _ap(ut[:], cig * 512, [ut[:].ap[0], [1, 1024]])
                nc.vector.tensor_tensor(out=outv, in0=v_v, in1=bc_v,
                                        op=Alu.mult)
                j = cig // 2
                nc.vector.tensor_tensor(
                    out=u2[ch][:, p * 1024 + j * 512:
                               p * 1024 + (j + 1) * 512],
                    in0=ut[:, cig, :], in1=ut[:, cig + 1, :], op=Alu.add)

    # ---------------- output projection: y^T = Wo u (+bo) --------------
    for h in range(2):
        for e in range(2):
            yp = pj.tile([P, 1024], f32, name="yp", tag="pj")
            for j in range(2):
                for ch in range(2):
                    lhsT = ws[ch][:, 3 * DM + e * P: 3 * DM + (e + 1) * P]
                    for pp in (2 * h, 2 * h + 1):
                        nc.tensor.matmul(
                            yp[:, (pp - 2 * h) * 512:(pp - 2 * h + 1) * 512],
                            lhsT=lhsT,
                            rhs=u2[ch][:, pp * 1024 + j * 512:
                                       pp * 1024 + (j + 1) * 512],
                            start=(ch == 0 and j == 0),
                            stop=(ch == 1 and j == 1))
            yo = yst.tile([P, 1024], f16, name="yo", tag="yo")
            nc.scalar.activation(yo[:], yp[:], Act.Identity,
                                 bias=bsb[e][:, 3:4])
            nc.scalar.dma_start(
                out=y_dram[e * P:(e + 1) * P, h * 1024:(h + 1) * 1024],
                in_=yo[:])


def build_nc():
    from contextlib import ExitStack
    nc = bacc.Bacc(trn_type="TRN2", target_bir_lowering=False, debug=False)
    d = {}
    for name in ("xq", "xk", "xv"):
        d[name] = nc.dram_tensor(name, [DM, ROWS], f16, kind="ExternalInput").ap()
    d["w"] = nc.dram_tensor("w", [DM, 4 * DM], f16, kind="ExternalInput").ap()
    d["bqkvo"] = nc.dram_tensor("bqkvo", [DM, 4], f32, kind="ExternalInput").ap()
    d["selkm"] = nc.dram_tensor("selkm", [DM, 224], f16, kind="ExternalInput").ap()
    d["selbc"] = nc.dram_tensor("selbc", [32, 8 * P], f16, kind="ExternalInput").ap()
    y = nc.dram_tensor("y", [DM, ROWS], f16, kind="ExternalOutput").ap()
    with tile.TileContext(nc) as tc:
        with ExitStack() as ctx:
            _emit(ctx, tc, nc, d, y)
    nc.compile()
    return nc


def make_shared_inputs(Wq, bq, Wk, bk, Wv, bv, Wo, bo):
    shared = {}
    wt = np.concatenate(
        [np.asarray(W, np.float32).T for W in (Wq, Wk, Wv, Wo)], axis=1)
    shared["w"] = wt.astype(np.float16)
    shared["bqkvo"] = np.ascontiguousarray(
        np.stack([bq, bk, bv, bo], axis=1), dtype=np.float32)
    # selkm[ch*128 + dd, 96 + 8*(dd//32) + 4*ch] = 0.5; score matmul for
    # pair p uses lhsT = selkm[ch][:, 96-p : 224-p] -> head hh of pair p
    # lands on psum lane 8*hh + 4*ch + p.
    selkm = np.zeros((DM, 224), np.float16)
    for ch in range(2):
        for dd in range(P):
            selkm[ch * P + dd, 96 + 8 * (dd // 32) + 4 * ch] = SCALE
    shared["selkm"] = selkm
    # selbc[k, (2p+ch)*128 + d] = 1 iff k == 8*(d//32) + 4*ch + p: the
    # broadcast matmul for (p, ch) maps p_sb lane 8*hh + 4*ch + p to
    # feature lanes 32*hh..32*hh+31 of channel half ch.
    selbc = np.zeros((32, 8 * P), np.float16)
    for p in range(NPAIR):
        for ch in range(2):
            for dd in range(P):
                selbc[8 * (dd // 32) + 4 * ch + p,
                      (2 * p + ch) * P + dd] = 1.0
    shared["selbc"] = selbc
    return shared


def make_core_inputs(query, key_in, value, core):
    # core i handles stock i: [4, 512, 256] -> feature-major [256, 2048]
    out = {}
    for name, x in (("xq", query), ("xk", key_in), ("xv", value)):
        xi = np.asarray(x[core], dtype=np.float32).reshape(ROWS, DM)
        out[name] = np.ascontiguousarray(xi.T).astype(np.float16)
    return out


def kernel(query, key_in, value, Wq, bq, Wk, bk, Wv, bv, Wo, bo):
    nc = build_nc()
    shared = make_shared_inputs(Wq, bq, Wk, bk, Wv, bv, Wo, bo)
    in_maps = []
    for core in range(NCORES):
        m = dict(shared)
        m.update(make_core_inputs(query, key_in, value, core))
        in_maps.append(m)
    res = run_bass_kernel_spmd(nc, in_maps, list(range(NCORES))).results
    # y dram is y^T [256, 2048]; host transposes back to [4, 512, 256]
    y = np.stack([
        np.ascontiguousarray(res[i]["y"].astype(np.float32).T).reshape(
            NB, S, DM)
        for i in range(NCORES)])
    return y.astype(np.float32)


# revision 62
# speedup vs baseline: 1.1319x; 1.0183x over previous
"""Trainium2 Bass kernel for sparse multi-headed attention.

Semantics (verified against the reference):
  q = x_q @ Wq.T + bq (per head, dk=32), same for k, v
  for each row s: attend to keys {s-c : c in (5,3,1,0), c <= s}
    score_c[s] = q[s].k[s-c] / sqrt(4)
    p = softmax over valid offsets
    attn[s] = sum_c p_c[s] * v[s-c]
  y = attn @ Wo.T + bo

Sharding: data-parallel over d_stock (8 stocks -> 8 cores). Each core
processes 4 (stock,batch) pairs = 2048 rows. Weights replicated.

v2 design (vs the tf32 baseline):
  - fp16 activations/weights on the wire and in matmuls (10-bit mantissa,
    ~tf32 accuracy, half the HBM traffic, fast PE weight loads), bf16 for
    everything post-exp (range safety: exp(score) can exceed fp16 max).
  - All DVE tensor-tensor ops use 16-bit dtypes -> 2x DVE mode.
  - The p head->feature broadcast runs on the DMA engines via a DRAM
    round-trip with a stride-0 source AP (frees the PE and keeps the
    attend multiplies in 2x mode), not on the PE.
  - Output is written feature-major (y^T) and transposed on the host.
  - Keep-alive matmuls bridge the softmax gap so the PE HAM clock gate
    never re-throttles to half speed.
"""

import numpy as np

from concourse import bacc, bass, mybir, tile
from concourse.bass_utils import run_bass_kernel_spmd

DS, NB, S, DM, H, DK = 8, 4, 512, 256, 8, 32
CONS = (5, 3, 1, 0)
NCORES = 8
NPAIR = NB  # pairs per core (1 stock x 4 batches)
ROWS = NPAIR * S  # 2048
P = 128
PADC = 8  # zero pad columns in front of k/v for shifted reads
NEG = -1e9
SCALE = 0.5  # 1/sqrt(n_att)

f32 = mybir.dt.float32
f16 = mybir.dt.float16
bf16 = mybir.dt.bfloat16
Act = mybir.ActivationFunctionType
Alu = mybir.AluOpType


def _ap(base, off_elems, dims):
    return bass.AP(tensor=base.tensor, offset=base.offset + off_elems, ap=dims)


def _emit(ctx, tc, nc, d, y_dram):
    main = ctx.enter_context(tc.tile_pool(name="main", bufs=1))
    prodp = ctx.enter_context(tc.tile_pool(name="prodp", bufs=4))
    utp = ctx.enter_context(tc.tile_pool(name="utp", bufs=8))
    yst = ctx.enter_context(tc.tile_pool(name="yst", bufs=2))
    pj = ctx.enter_context(tc.tile_pool(name="pj", bufs=2, space="PSUM"))
    scp = ctx.enter_context(tc.tile_pool(name="scp", bufs=1, space="PSUM"))
    dramp = ctx.enter_context(tc.tile_pool(name="dramp", bufs=1, space="DRAM"))

    # ---------------- input DMAs (SP queue, in issue order) ------------
    ws = [main.tile([P, 4 * DM], f16, name=f"ws{k}") for k in range(2)]
    for k in range(2):
        nc.sync.dma_start(out=ws[k][:], in_=d["w"][k * P:(k + 1) * P, :])
    bsb = [main.tile([P, 4], f32, name=f"bsb{ch}") for ch in range(2)]
    selk = [main.tile([P, 224], f16, name=f"selk{ch}") for ch in range(2)]
    selbc = main.tile([32, 8 * P], f16, name="selbc")
    xs = {}
    for name in ("xq", "xk", "xv"):
        for k in range(2):
            xs[name, k] = main.tile([P, ROWS], f16, name=f"{name}{k}")
    # halves of q/k first so projections start early; v afterwards.
    # Spread triggers over the two HWDGE queues so they don't serialize.
    for k in range(2):
        nc.sync.dma_start(out=xs["xq", k][:, 0:1024],
                          in_=d["xq"][k * P:(k + 1) * P, 0:1024])
        nc.scalar.dma_start(out=xs["xk", k][:, 0:1024],
                            in_=d["xk"][k * P:(k + 1) * P, 0:1024])
    for k in range(2):
        nc.sync.dma_start(out=xs["xq", k][:, 1024:2048],
                          in_=d["xq"][k * P:(k + 1) * P, 1024:2048])
        nc.scalar.dma_start(out=xs["xk", k][:, 1024:2048],
                            in_=d["xk"][k * P:(k + 1) * P, 1024:2048])
    nc.scalar.dma_start(out=selbc[:], in_=d["selbc"])
    for ch in range(2):
        nc.scalar.dma_start(out=bsb[ch][:],
                            in_=d["bqkvo"][ch * P:(ch + 1) * P, :])
        nc.scalar.dma_start(out=selk[ch][:],
                            in_=d["selkm"][ch * P:(ch + 1) * P, :])
    for k in range(2):
        nc.sync.dma_start(out=xs["xv", k][:],
                          in_=d["xv"][k * P:(k + 1) * P, :])

    # ---------------- PE warmup (HAM un-throttle) while DMAs run ------
    wtb = main.tile([P, 512], bf16, name="wtb")
    wtf = main.tile([P, 512], f16, name="wtf")
    nc.vector.memset(wtb[:], 0.0)
    nc.vector.memset(wtf[:], 0.0)
    # 6 x ~790ns cold crosses the 3.413us HAM window, so the PE
    # un-throttles before the projections (4 x was just short: the whole
    # front ran at 1.2GHz per the HAM event trace)
    for i in range(6):
        wps = pj.tile([P, 1024], f32, name="wps", tag="pj")
        nc.tensor.matmul(wps[:, 0:512], lhsT=wtb[:, 0:P], rhs=wtb[:],
                         start=True, stop=True)

    # ---------------- projections -------------------------------------
    q_sb = [main.tile([P, ROWS], f16, name=f"q{ch}") for ch in range(2)]
    k_sb = [main.tile([P, PADC + ROWS], f16, name=f"k{ch}") for ch in range(2)]
    v_sb = [main.tile([P, PADC + ROWS], bf16, name=f"v{ch}") for ch in range(2)]
    for ch in range(2):
        nc.vector.memset(k_sb[ch][:, 0:PADC], 0.0)
        nc.vector.memset(v_sb[ch][:, 0:PADC], 0.0)

    def project(name, bcol, ch, h):
        ps = pj.tile([P, 1024], f32, name="ps", tag="pj")
        for k in range(2):
            lhsT = ws[k][:, bcol * DM + ch * P: bcol * DM + (ch + 1) * P]
            for hh2 in range(2):  # fp16 moving operand max is 512
                nc.tensor.matmul(
                    ps[:, hh2 * 512:(hh2 + 1) * 512],
                    lhsT=lhsT,
                    rhs=xs[name, k][:, h * 1024 + hh2 * 512:
                                    h * 1024 + (hh2 + 1) * 512],
                    start=(k == 0), stop=(k == 1))
        # biases fold into the PSUM->SBUF copies: q/v on ACT (activation
        # bias), k on DVE (tensor_scalar add) to balance engine load
        bias_ap = bsb[ch][:, bcol:bcol + 1]
        if name == "xq":
            out = q_sb[ch][:, h * 1024:(h + 1) * 1024]
        elif name == "xk":
            out = k_sb[ch][:, PADC + h * 1024: PADC + (h + 1) * 1024]
        else:
            out = v_sb[ch][:, PADC + h * 1024: PADC + (h + 1) * 1024]
        if name != "xv" and h == 0:
            # half-column copies so pair-0 products start one copy sooner
            nc.scalar.activation(out[:, 0:512], ps[:, 0:512], Act.Identity,
                                 bias=bias_ap)
            nc.scalar.activation(out[:, 512:1024], ps[:, 512:1024],
                                 Act.Identity, bias=bias_ap)
        else:
            nc.scalar.activation(out, ps[:], Act.Identity, bias=bias_ap)

    for h in range(2):
        for ch in range(2):
            project("xq", 0, ch, h)
            project("xk", 1, ch, h)
    for h in range(2):
        for ch in range(2):
            project("xv", 2, ch, h)

    # ---------------- products (DVE) + scores (PE), cig-split ----------
    # pr[(p,ch,cig)][d, j, s] = q[d, p*512+s] * k[d, p*512+s-c(cig,j)]
    # sc[8*hh + 4*ch + p, ci*512 + s] = q_h . k_h(s-c) * 0.5  (h = 4ch+hh)
    # cig0 products/scores complete first so exp(half 0) overlaps the
    # cig1 score matmuls.
    sc = scp.tile([P, 4 * 512], f32, name="sc", tag="sc")
    p_sb = main.tile([P, 4, 512], bf16, name="p_sb")
    d2 = main.tile([P, 2, 512], bf16, name="d2")

    def products(cig, step):
        c_hi = CONS[cig]
        for p in range(NPAIR):
            for ch in range(2):
                pr = prodp.tile([P, 2, 512], f16, name="pr", tag="pr")
                q_ap = q_sb[ch][:, p * 512:(p + 1) * 512]
                q_b = _ap(q_ap, 0, [q_ap.ap[0], [0, 2], [1, 512]])
                k_ap = k_sb[ch][:]
                k_v = _ap(k_ap, PADC + p * 512 - c_hi,
                          [k_ap.ap[0], [step, 2], [1, 512]])
                # flat out view: a 3D out AP drops DVE to 1x mode
                pr_flat = _ap(pr[:], 0, [pr[:].ap[0], [1, 1024]])
                nc.vector.tensor_tensor(out=pr_flat, in0=k_v, in1=q_b,
                                        op=Alu.mult)
                prs[p, ch, cig] = pr

    def score_mms(cig):
        for p in range(NPAIR):
            for ch in range(2):
                lhsT = selk[ch][:, 96 - p: 224 - p]
                for j in range(2):
                    nc.tensor.matmul(
                        sc[:, (cig + j) * 512:(cig + j + 1) * 512],
                        lhsT=lhsT,
                        rhs=prs[p, ch, cig][:, j, :],
                        start=(p == 0 and ch == 0),
                        stop=(p == 3 and ch == 1))

    def exp_half(h):
        nc.scalar.activation(
            _ap(p_sb[:], h * 1024, [p_sb[:].ap[0], [1, 1024]]),
            sc[:, h * 1024:(h + 1) * 1024], Act.Exp)
        nc.vector.tensor_tensor(out=d2[:, h, :], in0=p_sb[:, 2 * h, :],
                                in1=p_sb[:, 2 * h + 1, :], op=Alu.add)

    prs = {}
    products(0, 2)
    score_mms(0)
    products(2, 1)  # before the memsets so the DVE doesn't stall on them
    for ci, c in ((0, 5), (1, 3)):
        nc.vector.memset(sc[:, ci * 512: ci * 512 + c], NEG)
    exp_half(0)
    score_mms(2)
    nc.vector.memset(sc[:, 2 * 512: 2 * 512 + 1], NEG)
    exp_half(1)

    # ---------------- softmax tail -------------------------------------
    den = main.tile([P, 512], f32, name="den")
    nc.vector.tensor_tensor(out=den[:], in0=d2[:, 0, :], in1=d2[:, 1, :],
                            op=Alu.add)
    rcpf = main.tile([P, 512], f32, name="rcpf")
    nc.vector.reciprocal_approx_fast(rcpf[:], den[:])
    rcp = main.tile([P, 512], bf16, name="rcp")
    nc.vector.tensor_copy(rcp[:], rcpf[:])
    for h in range(2):
        pv = _ap(p_sb[:], h * 1024, [p_sb[:].ap[0], [1, 1024]])
        pv3 = _ap(p_sb[:], h * 1024, [p_sb[:].ap[0], [512, 2], [1, 512]])
        r_b = _ap(rcp[:], 0, [rcp[:].ap[0], [0, 2], [1, 512]])
        nc.vector.tensor_tensor(out=pv, in0=pv3, in1=r_b, op=Alu.mult)

    # ---------------- p broadcast via DMA round-trip -------------------
    # pscr row 8*hh + 4*ch + p = sc lane; bc[d, ch*8192 + p*2048 + ci*512+s]
    # cig0 (offsets 5,3) is broadcast on the PE: one [32,128]-selector
    # matmul per (p, ch) with a partition-offset rhs slice of p_sb — no
    # DMA latency, attend starts right after pmult half 0.
    # cig1 (offsets 1,0) takes the DRAM round-trip, whose latency hides
    # under the cig0 attend work.
    pscr = dramp.tile([32, 1024], bf16, name="pscr")
    bc = main.tile([P, 16384], bf16, name="bc")
    pbase = pscr[:]
    nc.sync.dma_start(out=pscr[:], in_=p_sb[0:32, 2:4, :])
    for hh in range(4):
        src = _ap(pbase, hh * 8 * 1024,
                  [[0, 32], [1024, 8], [1, 1024]])
        dst_base = bc[32 * hh:32 * (hh + 1), :]
        dst = _ap(dst_base, 1024,
                  [dst_base.ap[0], [2048, 8], [1, 1024]])
        # split across the two HWDGE rings so transfers run in parallel
        eng = nc.sync if hh < 2 else nc.scalar
        eng.dma_start(out=dst, in_=src)
    # dense bridge right after the scores: ~3.5us of continuous dummy
    # matmuls carries the HAM clock gate through the softmax latency
    # (the gate dropped at scores-end + 3.4us in the best trace)
    for i in range(16):
        wps = pj.tile([P, 1024], f32, name="wps", tag="pj")
        nc.tensor.matmul(wps[:, 0:512], lhsT=wtb[:, 0:P], rhs=wtb[:],
                         start=True, stop=True)
    # keep-alives so the HAM clock gate stays open through the softmax
    for rhs in (p_sb[:, 0, :], p_sb[:, 2, :], d2[:, 0, :]):
        ka = pj.tile([P, 1024], f32, name="ka", tag="pj")
        nc.tensor.matmul(ka[:, 0:512], lhsT=wtb[:, 0:P], rhs=rhs,
                         start=True, stop=True)
    # cig0 broadcast lands in PSUM, then ACT (idle here) copies it to
    # SBUF bf16 so the attend multiplies run in the DVE 2x mode.
    bc_ps = {}
    for p in range(NPAIR):
        for ch in range(2):
            bp = pj.tile([P, 1024], f32, name="bp", tag="pj")
            idx = 2 * p + ch
            for ci in range(2):
                nc.tensor.matmul(bp[:, ci * 512:(ci + 1) * 512],
                                 lhsT=selbc[:, idx * P:(idx + 1) * P],
                                 rhs=p_sb[0:32, ci, :],
                                 start=True, stop=True)
            bc_ps[p, ch] = bp

    # ---------------- attend (DVE, 16-bit) -----------------------------
    # ut[ci] = v[d, s-c] * bc;  u2 col p*1024 + j*512 + s, j = cig half
    u2 = [main.tile([P, 4096], f16, name=f"u2{ch}") for ch in range(2)]
    uts = {}
    for cig, step in ((0, 2), (2, 1)):
        c_hi = CONS[cig]
        for p in range(NPAIR):
            for ch in range(2):
                if cig == 0:
                    uts[p, ch] = utp.tile([P, 4, 512], bf16, name="ut",
                                          tag="ut")
                ut = uts[p, ch]
                if cig == 0:
                    bc_v = bc_ps[p, ch][:]
                else:
                    bc_v = _ap(bc[:], ch * 8192 + p * 2048 + cig * 512,
                               [bc[:].ap[0], [512, 2], [1, 512]])
                v_ap = v_sb[ch][:]
                v_v = _ap(v_ap, PADC + p * 512 - c_hi,
                          [v_ap.ap[0], [step, 2], [1, 512]])
                outv = _ap(ut[:], cig * 512, [ut[:].ap[0], [1, 1024]])
                nc.vector.tensor_tensor(out=outv, in0=v_v, in1=bc_v,
                                        op=Alu.mult)
                j = cig // 2
                nc.vector.tensor_tensor(
                    out=u2[ch][:, p * 1024 + j * 512:
                               p * 1024 + (j + 1) * 512],
                    in0=ut[:, cig, :], in1=ut[:, cig + 1, :], op=Alu.add)

    # ---------------- output projection: y^T = Wo u (+bo) --------------
    for h in range(2):
        for e in range(2):
            yp = pj.tile([P, 1024], f32, name="yp", tag="pj")
            for j in range(2):
                for ch in range(2):
                    lhsT = ws[ch][:, 3 * DM + e * P: 3 * DM + (e + 1) * P]
                    for pp in (2 * h, 2 * h + 1):
                        nc.tensor.matmul(
                            yp[:, (pp - 2 * h) * 512:(pp - 2 * h + 1) * 512],
                            lhsT=lhsT,
                            rhs=u2[ch][:, pp * 1024 + j * 512:
                                       pp * 1024 + (j + 1) * 512],
                            start=(ch == 0 and j == 0),
                            stop=(ch == 1 and j == 1))
            yo = yst.tile([P, 1024], f16, name="yo", tag="yo")
            nc.scalar.activation(yo[:], yp[:], Act.Identity,
                                 bias=bsb[e][:, 3:4])
            # trigger on the (idle) SP queue so it doesn't sit between
            # the final ACT copies
            nc.sync.dma_start(
                out=y_dram[e * P:(e + 1) * P, h * 1024:(h + 1) * 1024],
                in_=yo[:])


def build_nc():
    from contextlib import ExitStack
    nc = bacc.Bacc(trn_type="TRN2", target_bir_lowering=False, debug=False)
    d = {}
    for name in ("xq", "xk", "xv"):
        d[name] = nc.dram_tensor(name, [DM, ROWS], f16, kind="ExternalInput").ap()
    d["w"] = nc.dram_tensor("w", [DM, 4 * DM], f16, kind="ExternalInput").ap()
    d["bqkvo"] = nc.dram_tensor("bqkvo", [DM, 4], f32, kind="ExternalInput").ap()
    d["selkm"] = nc.dram_tensor("selkm", [DM, 224], f16, kind="ExternalInput").ap()
    d["selbc"] = nc.dram_tensor("selbc", [32, 8 * P], f16, kind="ExternalInput").ap()
    y = nc.dram_tensor("y", [DM, ROWS], f16, kind="ExternalOutput").ap()
    with tile.TileContext(nc) as tc:
        with ExitStack() as ctx:
            _emit(ctx, tc, nc, d, y)
    nc.compile()
    return nc


def make_shared_inputs(Wq, bq, Wk, bk, Wv, bv, Wo, bo):
    shared = {}
    wt = np.concatenate(
        [np.asarray(W, np.float32).T for W in (Wq, Wk, Wv, Wo)], axis=1)
    shared["w"] = wt.astype(np.float16)
    shared["bqkvo"] = np.ascontiguousarray(
        np.stack([bq, bk, bv, bo], axis=1), dtype=np.float32)
    # selkm[ch*128 + dd, 96 + 8*(dd//32) + 4*ch] = 0.5; score matmul for
    # pair p uses lhsT = selkm[ch][:, 96-p : 224-p] -> head hh of pair p
    # lands on psum lane 8*hh + 4*ch + p.
    selkm = np.zeros((DM, 224), np.float16)
    for ch in range(2):
        for dd in range(P):
            selkm[ch * P + dd, 96 + 8 * (dd // 32) + 4 * ch] = SCALE
    shared["selkm"] = selkm
    # selbc[k, (2p+ch)*128 + d] = 1 iff k == 8*(d//32) + 4*ch + p: the
    # broadcast matmul for (p, ch) maps p_sb lane 8*hh + 4*ch + p to
    # feature lanes 32*hh..32*hh+31 of channel half ch.
    selbc = np.zeros((32, 8 * P), np.float16)
    for p in range(NPAIR):
        for ch in range(2):
            for dd in range(P):
                selbc[8 * (dd // 32) + 4 * ch + p,
                      (2 * p + ch) * P + dd] = 1.0
    shared["selbc"] = selbc
    return shared


def make_core_inputs(query, key_in, value, core):
    # core i handles stock i: [4, 512, 256] -> feature-major [256, 2048]
    out = {}
    for name, x in (("xq", query), ("xk", key_in), ("xv", value)):
        xi = np.asarray(x[core], dtype=np.float32).reshape(ROWS, DM)
        out[name] = np.ascontiguousarray(xi.T).astype(np.float16)
    return out


def kernel(query, key_in, value, Wq, bq, Wk, bk, Wv, bv, Wo, bo):
    nc = build_nc()
    shared = make_shared_inputs(Wq, bq, Wk, bk, Wv, bv, Wo, bo)
    in_maps = []
    for core in range(NCORES):
        m = dict(shared)
        m.update(make_core_inputs(query, key_in, value, core))
        in_maps.append(m)
    res = run_bass_kernel_spmd(nc, in_maps, list(range(NCORES))).results
    # y dram is y^T [256, 2048]; host transposes back to [4, 512, 256]
    y = np.stack([
        np.ascontiguousarray(res[i]["y"].astype(np.float32).T).reshape(
            NB, S, DM)
        for i in range(NCORES)])
    return y.astype(np.float32)
